# revision 16
# baseline (speedup 1.0000x reference)
"""CARAFE++ content-aware upsampling kernel for Trainium2 (8 NeuronCores), v6.

Per-core pipeline (4 batches x 2 row-halves):
  1. conv1 as matmul (fp16) + relu -> featd: W-padded feat in partitions 0-63,
     one-row-shifted copy in partitions 64-127 (for conv2 row-pair packing)
  2. conv2 as 6 shifted matmuls per 8-row tile (3x K=128 row-pairs + 3x K=64),
     + bias + exp -> wkn (raw, un-normalized)
  3. XBAR dma-transpose wkn -> wkT (pixel-major) IMMEDIATELY after each exp;
     softmax normalization happens after the transpose, in pixel-major
     layout: per conv tile one [128,16] sums matmul group + one small
     reciprocal, then per block 4 tensor_scalar_mul -> wnb (scatter feed)
  4. per block: gpsimd local_scatter builds the band-matrix-transpose layout;
     per block-pair ONE XBAR dma-transpose yields 24 S panels; 12 accumulated
     fp16 matmuls reassemble; contiguous fp16 evictions (host does the
     pixel-shuffle) + grouped stores
All XBAR transposes serialize on the sync HWDGE ring (HW hazard otherwise);
loads/stores ride the scalar ring. XBAR writes/reads get manual deps (Tile
does not track InstDmaTransposeAnt). wkT transposes are sandwiched into the
T2 stream with explicit order pins so the scheduler cannot stall it.
"""
import sys

sys.path.insert(0, "/opt/trn_rl_repo")

import numpy as np
from contextlib import ExitStack

import concourse.bass as bass
import concourse.bacc as bacc
import concourse.tile as tile
from concourse.tile import add_dep_helper
from concourse import mybir
from concourse.bass_utils import run_bass_kernel_spmd

B, C, H, W = 4, 256, 64, 64
SCALE, K, COMP, G = 2, 5, 4, 1
MID = 64
ENC = 100          # K*K*SCALE*SCALE
NROW = 36          # x rows per core (32 + 2 halo each side)
NPX = NROW * W     # 2304
FROW = 34          # feat rows r0-1 .. r0+32
FPW = W + 2        # 66, feat row W-padded
NBLK = 16          # output row-pair blocks per core
NJB = 18           # x row-pair panels per core
XSPLIT = 1600      # x column split point per half (covers conv1 tiles 0-2)

f32 = mybir.dt.float32
f16 = mybir.dt.float16
i16 = mybir.dt.int16

# const_f16 blob column layout
C_WC0, C_WC1, C_WEP, C_WES, C_ONES, C_E4 = 0, 64, 128, 428, 728, 732
C_TOT = 832

_CACHE = {}


def _build_idxs():
    """Per-partition scatter indices. Partition = out-center px (rt, w); slot
    = (p, dy, dx) wk channel order; dest = (dj*4+p)*128 + rb*64 + wi so the
    XBAR panel-major transpose yields S panels grouped (dj, p)."""
    idxs = np.full((128, 100), -1, np.int16)
    for rt in range(2):
        for w in range(W):
            part = rt * W + w
            for p in range(4):
                for dy in range(-2, 3):
                    dj = (rt + dy + 2) // 2
                    rb = (rt + dy) % 2
                    for dx in range(-2, 3):
                        wi = w + dx
                        if 0 <= wi < W:
                            slot = p * 25 + (dy + 2) * 5 + (dx + 2)
                            idxs[part, slot] = (dj * 4 + p) * 128 + rb * 64 + wi
    return idxs


def _build_nc():
    nc = bacc.Bacc("TRN2", target_bir_lowering=False, debug=False, num_devices=8)

    # ---- DRAM I/O (per-core shapes)
    d_x = nc.dram_tensor("x", [128, 2 * NPX], f16, kind="ExternalInput")
    d_xt = nc.dram_tensor("xt", [128, NJB * 2 * 128], f16, kind="ExternalInput")
    d_cst = nc.dram_tensor("cst", [128, C_TOT], f16, kind="ExternalInput")
    d_cstf = nc.dram_tensor("cstf", [128, 2], f32, kind="ExternalInput")
    d_idx = nc.dram_tensor("idx", [128, 100], i16, kind="ExternalInput")
    d_out = nc.dram_tensor("out", [128, 2 * NBLK * 512], f16,
                           kind="ExternalOutput")

    with tile.TileContext(nc) as tc, ExitStack() as ctx:
        sb1 = ctx.enter_context(tc.tile_pool(name="sb1", bufs=1))
        sbw = ctx.enter_context(tc.tile_pool(name="sbw", bufs=2))
        ps = ctx.enter_context(tc.tile_pool(name="ps", bufs=1, space="PSUM"))

        # ---- loads. scalar ring: weights first, then x halves (conv1 tile 0
        # only gates on the "a" chunks); sync ring: xt (consumed late).
        cst = sb1.tile([128, C_TOT], f16, tag="cst")
        cstf = sb1.tile([128, 2], f32, tag="cstf")
        sidx = sb1.tile([128, 100], i16, tag="sidx")
        xh = []   # [h][chunk] tiles
        nc.scalar.dma_start(out=cst, in_=d_cst[:])
        for h in range(2):
            a = sb1.tile([128, XSPLIT], f16, tag=f"xh{h}a")
            b = sb1.tile([128, NPX - XSPLIT], f16, tag=f"xh{h}b")
            xh.append((a, b))
        for h in range(2):
            nc.scalar.dma_start(out=xh[h][0],
                                in_=d_x[:, h * NPX:h * NPX + XSPLIT])
        nc.scalar.dma_start(out=cstf, in_=d_cstf[:])
        nc.scalar.dma_start(out=sidx, in_=d_idx[:])
        for h in range(2):
            nc.scalar.dma_start(out=xh[h][1],
                                in_=d_x[:, h * NPX + XSPLIT:(h + 1) * NPX])
        xt = sb1.tile([128, NJB, 2, 128], f16, tag="xt")
        nc.sync.dma_start(
            out=xt, in_=d_xt[:].rearrange("q (j c m) -> q j c m", j=NJB, c=2))

        bc = cstf[0:MID, 0:1]
        be = cstf[0:ENC, 1:2]
        wc0 = cst[:, C_WC0:C_WC0 + 64]
        wc1 = cst[:, C_WC1:C_WC1 + 64]
        ones = cst[0:ENC, C_ONES:C_ONES + 4]

        # warm the Exp activation table during load shadow
        scratch = sb1.tile([ENC, 1], f32, tag="scratch")
        nc.scalar.activation(out=scratch[:], in_=be,
                             func=mybir.ActivationFunctionType.Exp,
                             bias=be, scale=1.0)

        featd = sb1.tile([128, FROW * FPW], f16, tag="featd")
        nc.vector.memset(featd, 0.0)
        wkn = sb1.tile([112, 2048], f16, tag="wkn")
        nc.vector.memset(wkn[96:112, :], 0.0)
        wkTs = [None] * 4   # per conv tile [128, 4, 112] raw transposed exp
        wkT_x = [None] * 4  # transpose instr per conv tile
        exp_is = [None] * 4
        wnbs = [None] * NBLK   # per block normalized scatter feed [128, 100]

        def xsrc(h, n0, n):
            """x slice [n0, n0+n) of half h from the right chunk tile."""
            if n0 + n <= XSPLIT:
                return xh[h][0][:, n0:n0 + n]
            return xh[h][1][:, n0 - XSPLIT:n0 - XSPLIT + n]

        # ---- conv1 tile: 1x1 conv (256->64) + relu -> featd (both halves)
        def conv1_tile(nt):
            n0 = W + nt * 512
            n = min(512, 2240 - n0)
            pf = ps.tile([MID, 512], f32, tag="pf", bufs=2, name="pf")
            nc.tensor.matmul(pf[:, :n], wc0, xsrc(0, n0, n),
                             start=True, stop=False)
            nc.tensor.matmul(pf[:, :n], wc1, xsrc(1, n0, n),
                             start=False, stop=True)
            fp0 = n0 // W - 1
            nr = n // W
            src = pf[:, :n].rearrange("m (r w) -> m r w", w=W)
            fd1h = featd[0:64]
            dst1 = bass.AP(
                tensor=fd1h.tensor, offset=fd1h.offset + fp0 * FPW + 1,
                ap=[fd1h.ap[0], [FPW, nr], [1, W]],
            )
            nc.scalar.activation(out=dst1, in_=src,
                                 func=mybir.ActivationFunctionType.Relu,
                                 bias=bc, scale=1.0)
            fd2 = featd[64:128]
            if fp0 == 0:
                src2 = bass.AP(tensor=pf.tensor, offset=pf.offset + W,
                               ap=[pf.ap[0], [W, nr - 1], [1, W]])
                dst2 = bass.AP(tensor=fd2.tensor, offset=fd2.offset + 1,
                               ap=[fd2.ap[0], [FPW, nr - 1], [1, W]])
            else:
                src2 = bass.AP(tensor=pf.tensor, offset=pf.offset,
                               ap=[pf.ap[0], [W, nr], [1, W]])
                dst2 = bass.AP(tensor=fd2.tensor,
                               offset=fd2.offset + (fp0 - 1) * FPW + 1,
                               ap=[fd2.ap[0], [FPW, nr], [1, W]])
            nc.scalar.activation(out=dst2, in_=src2,
                                 func=mybir.ActivationFunctionType.Relu,
                                 bias=bc, scale=1.0)

        # ---- conv2: 6 matmuls per 8-row tile, + bias + exp -> wkn
        def conv2_mm(nt):
            h0 = nt * 8
            pw = ps.tile([ENC, 512], f32, tag="pw", bufs=1, name="pw")
            for j in range(3):
                rhs = bass.AP(
                    tensor=featd.tensor, offset=featd.offset + h0 * FPW + j,
                    ap=[featd.ap[0], [FPW, 8], [1, W]],
                )
                nc.tensor.matmul(pw[:], cst[:, C_WEP + j * ENC:C_WEP + (j + 1) * ENC],
                                 rhs, start=(j == 0), stop=False)
            fd1 = featd[0:64]
            for j in range(3):
                rhs = bass.AP(
                    tensor=fd1.tensor,
                    offset=fd1.offset + (h0 + 2) * FPW + j,
                    ap=[fd1.ap[0], [FPW, 8], [1, W]],
                )
                nc.tensor.matmul(pw[:], cst[0:64, C_WES + j * ENC:C_WES + (j + 1) * ENC],
                                 rhs, start=False, stop=(j == 2))
            return pw

        def conv2_exp(nt, pw):
            sl = slice(nt * 512, (nt + 1) * 512)
            e = nc.scalar.activation(out=wkn[0:ENC, sl], in_=pw[:],
                                     func=mybir.ActivationFunctionType.Exp,
                                     bias=be, scale=1.0)
            exp_is[nt] = e

        def wkt_x(nt):
            """XBAR transpose raw exp (wkn) -> wkT for conv tile nt."""
            sl = slice(nt * 512, (nt + 1) * 512)
            wkT = sbw.tile([128, 4, 112], f16, tag="wkT", bufs=4, name="wkT")
            wkTs[nt] = wkT
            wx = nc.sync.dma_start_transpose(out=wkT[:], in_=wkn[:, sl])
            # Tile does not dep-track XBAR transposes: manual edge
            add_dep_helper(wx.ins, exp_is[nt].ins, reason="xbar wkT reads wkn")
            wkT_x[nt] = wx
            return wx

        def tile_norm(nt, wx):
            """Post-transpose softmax normalization for conv tile nt: one
            [128,16] sums group + one reciprocal + 16 tensor_scalar_mul."""
            pa = ps.tile([128, 16], f32, tag="pa", bufs=1, name="pa")
            for b in range(4):
                t = nt * 4 + b
                nc.tensor.matmul(pa[:, b * 4:(b + 1) * 4],
                                 wkn[0:ENC, t * 128:(t + 1) * 128], ones,
                                 start=True, stop=True)
            rT = sbw.tile([128, 16], f32, tag="rT", bufs=2, name="rT")
            nc.vector.reciprocal(rT[:], pa[:])
            for b in range(4):
                t = nt * 4 + b
                wnb = sbw.tile([128, 100], f16, tag="wnb", bufs=12, name="wnb")
                wnbs[t] = wnb
                for p in range(4):
                    ts = nc.vector.tensor_scalar_mul(
                        wnb[:, p * 25:(p + 1) * 25],
                        wkTs[nt][:, b, p * 25:p * 25 + 25],
                        rT[:, b * 4 + p:b * 4 + p + 1])
                    if b == 0 and p == 0:
                        add_dep_helper(ts.ins, wx.ins,
                                       reason="norm reads xbar wkT")

        # ---- conv phase emission
        conv1_tile(0)
        conv1_tile(1)
        pw0 = conv2_mm(0)
        conv2_exp(0, pw0)
        conv1_tile(2)
        w0 = wkt_x(0)
        tile_norm(0, w0)
        pw1 = conv2_mm(1)
        conv2_exp(1, pw1)
        conv1_tile(3)
        w1 = wkt_x(1)
        tile_norm(1, w1)
        pw2 = conv2_mm(2)
        conv2_exp(2, pw2)
        conv1_tile(4)
        w2 = wkt_x(2)
        tile_norm(2, w2)
        pw3 = conv2_mm(3)
        conv2_exp(3, pw3)
        w3 = wkt_x(3)
        tile_norm(3, w3)

        # ---- reassembly: per block scatter; per pair one XBAR transpose,
        # 12 matmuls, contiguous fp16 evictions, grouped stores.
        sdst_reader = [None] * 4
        t2list = [None] * 8
        osegs = [None] * 4

        def scatter(t):
            g = t // 2
            if t % 2 == 0:
                sd = sbw.tile([128, 2, 1536], f16, tag="sdst", bufs=4, name="sd")
                t2list[g] = sd
            sd = t2list[g]
            sc = nc.gpsimd.local_scatter(
                out_ap=sd[:, t % 2, :], data_ap=wnbs[t][:],
                idxs_ap=sidx[:],
                channels=128, num_elems=1536, num_idxs=100,
            )
            if sdst_reader[g % 4] is not None:
                add_dep_helper(sc.ins, sdst_reader[g % 4].ins,
                               reason="WAR: scatter overwrites xbar-read sdst")
            return sc

        def transpose_pair(g, sc0, sc1):
            T2 = sbw.tile([128, 24, 128], f16, tag="T2", bufs=4, name="T2")
            tx = nc.sync.dma_start_transpose(out=T2[:], in_=t2list[g][:])
            add_dep_helper(tx.ins, sc0.ins, reason="xbar reads scatter0")
            add_dep_helper(tx.ins, sc1.ins, reason="xbar reads scatter1")
            sdst_reader[g % 4] = tx
            return T2, tx

        def reassemble(t, T2, tx):
            for ch in range(2):
                po = ps.tile([128, 512], f32, tag="po", bufs=4, name="po")
                for dj in range(3):
                    pan = (t % 2) * 12 + dj * 4
                    mm_i = nc.tensor.matmul(
                        po[:], xt[:, t + dj, ch, :], T2[:, pan:pan + 4, :],
                        start=(dj == 0), stop=(dj == 2),
                    )
                    if ch == 0 and dj == 0:
                        add_dep_helper(mm_i.ins, tx.ins, reason="PE reads xbar T")
                if t % 4 == 0 and ch == 0:
                    osegs[(t // 4)] = sbw.tile([128, 8, 512], f16, tag="oseg",
                                               bufs=2, name="oseg")
                dst = osegs[t // 4][:, (t % 4) * 2 + ch, :]
                if ch == 0:
                    nc.scalar.activation(out=dst, in_=po[:],
                                         func=mybir.ActivationFunctionType.Copy,
                                         scale=1.0)
                else:
                    nc.vector.tensor_copy(dst, po[:])
            last = mm_i
            if t % 4 == 3:
                k = t // 4
                nc.scalar.dma_start(
                    out=d_out[:, k * 4096:(k + 1) * 4096],
                    in_=osegs[k][:],
                )
            return last

        # software-pipelined emission: pair g+1's scatters+transpose are
        # emitted before pair g's matmuls. wkT2/wkT3 are sandwiched into the
        # T2 stream with order pins so the scheduler cannot misplace them.
        pend = {}
        last_pe = [None] * 4   # T2 slot -> last PE reader
        sc0 = scatter(0)
        sc1 = scatter(1)
        pend[0] = transpose_pair(0, sc0, sc1)
        # early wkT transposes must precede the T2 stream on the sync ring
        add_dep_helper(pend[0][1].ins, w0.ins, reason="order T2x0 after wkT0")
        add_dep_helper(pend[0][1].ins, w1.ins, reason="order T2x0 after wkT1")
        # sandwich wkT2 between T2x0 and T2x1, wkT3 between T2x2 and T2x3
        add_dep_helper(w2.ins, pend[0][1].ins, reason="order wkT2 after T2x0")
        for g in range(8):
            if g + 1 < 8:
                sc0 = scatter(2 * g + 2)
                sc1 = scatter(2 * g + 3)
                pend[g + 1] = transpose_pair(g + 1, sc0, sc1)
                if g + 1 == 1:
                    add_dep_helper(pend[1][1].ins, w2.ins,
                                   reason="order T2x1 after wkT2")
                if g + 1 == 2:
                    add_dep_helper(w3.ins, pend[2][1].ins,
                                   reason="order wkT3 after T2x2")
                if g + 1 == 3:
                    add_dep_helper(pend[3][1].ins, w3.ins,
                                   reason="order T2x3 after wkT3")
                if last_pe[(g + 1) % 4] is not None:
                    add_dep_helper(pend[g + 1][1].ins, last_pe[(g + 1) % 4].ins,
                                   reason="WAR: xbar overwrites PE-read T2")
            T2, tx = pend[g]
            reassemble(2 * g, T2, tx)
            last_pe[g % 4] = reassemble(2 * g + 1, T2, tx)

    nc.compile()
    return nc


def _host_prep(x, W_comp, b_comp, W_enc, b_enc):
    """Build per-core input maps (layout/dtype prep only)."""
    idxs = _build_idxs()
    cst = np.zeros((128, C_TOT), np.float16)
    cst[0:128, C_WC0:C_WC0 + 64] = W_comp.T[0:128]
    cst[0:128, C_WC1:C_WC1 + 64] = W_comp.T[128:256]
    for j in range(3):
        cst[0:64, C_WEP + j * ENC:C_WEP + (j + 1) * ENC] = W_enc[:, :, 0, j].T
        cst[64:128, C_WEP + j * ENC:C_WEP + (j + 1) * ENC] = W_enc[:, :, 1, j].T
        cst[0:64, C_WES + j * ENC:C_WES + (j + 1) * ENC] = W_enc[:, :, 2, j].T
    for p in range(4):
        cst[C_ONES * 0 + p * 25:(p + 1) * 25, C_ONES + p] = 1.0
        cst[p, C_E4 + p * 25:C_E4 + (p + 1) * 25] = 1.0
    cstf = np.zeros((128, 2), np.float32)
    cstf[0:MID, 0] = b_comp
    cstf[0:ENC, 1] = b_enc

    xp = np.pad(x, ((0, 0), (0, 0), (2, 2), (0, 0)))   # (B, C, 68, 64)
    in_maps = []
    for core in range(8):
        b, half = core // 2, core % 2
        r0 = 32 * half
        xs = np.ascontiguousarray(
            xp[b, :, r0:r0 + NROW, :].reshape(C, NPX)).astype(np.float16)
        xflat = np.ascontiguousarray(xs.reshape(2, 128, NPX).transpose(1, 0, 2)
                                     ).reshape(128, 2 * NPX)
        xtc = np.ascontiguousarray(
            xs.reshape(2, 128, NJB, 128).transpose(3, 2, 0, 1)
        ).reshape(128, NJB * 2 * 128)
        in_maps.append(dict(x=xflat, xt=xtc, cst=cst, cstf=cstf, idx=idxs))
    return in_maps


def _gather(res):
    """Assemble full (B, C, 128, 128) fp32 output from per-core raw stores.

    Per core: out [128, 32, 512] f16 where free = (t, ch2, p, rt, w) with
    po free = (p, rt*64+w). Output pixel-shuffle: input row i = 2t+rt of the
    half; out rows 2i + (4w+p)//128, col (4w+p) % 128.
    """
    out = np.empty((B, C, 128, 128), np.float32)
    for core in range(8):
        b, half = core // 2, core % 2
        a = res.results[core]["out"].astype(np.float32)
        a = a.reshape(128, 16, 2, 4, 2, 64)      # (cp, t, c2, p, rt, w)
        a = a.transpose(1, 4, 2, 0, 5, 3)        # (t, rt, c2, cp, w, p)
        a = a.reshape(16, 2, 256, 2, 128)        # (t, rt, ch, rowbit, col)
        a = a.transpose(2, 0, 1, 3, 4).reshape(256, 64, 128)
        out[b, :, 64 * half:64 * (half + 1), :] = a
    return out


def kernel(x, W_comp, b_comp, W_enc, b_enc):
    x = np.asarray(x, np.float32)
    W_comp = np.asarray(W_comp, np.float32)
    b_comp = np.asarray(b_comp, np.float32)
    W_enc = np.asarray(W_enc, np.float32)
    b_enc = np.asarray(b_enc, np.float32)

    if "nc" not in _CACHE:
        _CACHE["nc"] = _build_nc()
    nc = _CACHE["nc"]

    in_maps = _host_prep(x, W_comp, b_comp, W_enc, b_enc)
    res = run_bass_kernel_spmd(nc, in_maps, core_ids=list(range(8)))
    return _gather(res)


if __name__ == "__main__":
    d = np.load("/tmp/carafe_ref.npz")
    expected = d["expected"]
    out = kernel(**{k: d[k] for k in ["x", "W_comp", "b_comp", "W_enc", "b_enc"]})
    err = np.abs(out - expected)
    scale = np.abs(expected).max()
    print(f"absmax err: {err.max():.4e}  rel: {err.max()/scale:.4e}")


# revision 21
# speedup vs baseline: 1.0698x; 1.0698x over previous
"""CARAFE++ content-aware upsampling kernel for Trainium2 (8 NeuronCores), v6.

Per-core pipeline (4 batches x 2 row-halves):
  1. conv1 as matmul (fp16) + relu -> featd: W-padded feat in partitions 0-63,
     one-row-shifted copy in partitions 64-127 (for conv2 row-pair packing)
  2. conv2 as 6 shifted matmuls per 8-row tile (3x K=128 row-pairs + 3x K=64),
     + bias + exp -> wkn (raw, un-normalized)
  3. XBAR dma-transpose wkn -> wkT (pixel-major) IMMEDIATELY after each exp;
     softmax normalization happens after the transpose, in pixel-major
     layout: per conv tile one [128,16] sums matmul group + one small
     reciprocal, then per block 4 tensor_scalar_mul -> wnb (scatter feed)
  4. per block: gpsimd local_scatter builds the band-matrix-transpose layout;
     per block-pair ONE XBAR dma-transpose yields 24 S panels; 12 accumulated
     fp16 matmuls reassemble; contiguous fp16 evictions (host does the
     pixel-shuffle) + grouped stores
All XBAR transposes serialize on the sync HWDGE ring (HW hazard otherwise);
loads/stores ride the scalar ring. XBAR writes/reads get manual deps (Tile
does not track InstDmaTransposeAnt). wkT transposes are sandwiched into the
T2 stream with explicit order pins so the scheduler cannot stall it.
"""
import sys

sys.path.insert(0, "/opt/trn_rl_repo")

import numpy as np
from contextlib import ExitStack

import concourse.bass as bass
import concourse.bacc as bacc
import concourse.tile as tile
from concourse.tile import add_dep_helper
from concourse import mybir
from concourse.bass_utils import run_bass_kernel_spmd

B, C, H, W = 4, 256, 64, 64
SCALE, K, COMP, G = 2, 5, 4, 1
MID = 64
ENC = 100          # K*K*SCALE*SCALE
NROW = 36          # x rows per core (32 + 2 halo each side)
NPX = NROW * W     # 2304
FROW = 34          # feat rows r0-1 .. r0+32
FPW = W + 2        # 66, feat row W-padded
NBLK = 16          # output row-pair blocks per core
NJB = 18           # x row-pair panels per core
XSPLIT = 1600      # x column split point per half (covers conv1 tiles 0-2)

f32 = mybir.dt.float32
f16 = mybir.dt.float16
i16 = mybir.dt.int16

# const_f16 blob column layout
C_WC0, C_WC1, C_WEP, C_WES, C_ONES, C_E4 = 0, 64, 128, 428, 728, 732
C_TOT = 832

_CACHE = {}


def _build_idxs():
    """Per-partition scatter indices. Partition = out-center px (rt, w); slot
    = (p, dy, dx) wk channel order; dest = (dj*4+p)*128 + rb*64 + wi so the
    XBAR panel-major transpose yields S panels grouped (dj, p)."""
    idxs = np.full((128, 100), -1, np.int16)
    for rt in range(2):
        for w in range(W):
            part = rt * W + w
            for p in range(4):
                for dy in range(-2, 3):
                    dj = (rt + dy + 2) // 2
                    rb = (rt + dy) % 2
                    for dx in range(-2, 3):
                        wi = w + dx
                        if 0 <= wi < W:
                            slot = p * 25 + (dy + 2) * 5 + (dx + 2)
                            idxs[part, slot] = (dj * 4 + p) * 128 + rb * 64 + wi
    return idxs


def _build_nc():
    nc = bacc.Bacc("TRN2", target_bir_lowering=False, debug=False, num_devices=8)

    # ---- DRAM I/O (per-core shapes)
    d_x = nc.dram_tensor("x", [128, 2 * NPX], f16, kind="ExternalInput")
    d_xt = nc.dram_tensor("xt", [128, NJB * 2 * 128], f16, kind="ExternalInput")
    d_cst = nc.dram_tensor("cst", [128, C_TOT], f16, kind="ExternalInput")
    d_cstf = nc.dram_tensor("cstf", [128, 2], f32, kind="ExternalInput")
    d_idx = nc.dram_tensor("idx", [128, 100], i16, kind="ExternalInput")
    d_out = nc.dram_tensor("out", [128, 2 * NBLK * 512], f16,
                           kind="ExternalOutput")

    with tile.TileContext(nc) as tc, ExitStack() as ctx:
        sb1 = ctx.enter_context(tc.tile_pool(name="sb1", bufs=1))
        sbw = ctx.enter_context(tc.tile_pool(name="sbw", bufs=2))
        ps = ctx.enter_context(tc.tile_pool(name="ps", bufs=1, space="PSUM"))

        # ---- loads. scalar ring: weights first, then x halves (conv1 tile 0
        # only gates on the "a" chunks); sync ring: xt (consumed late).
        cst = sb1.tile([128, C_TOT], f16, tag="cst")
        cstf = sb1.tile([128, 2], f32, tag="cstf")
        sidx = sb1.tile([128, 100], i16, tag="sidx")
        nc.scalar.dma_start(out=cst, in_=d_cst[:])
        nc.scalar.dma_start(out=cstf, in_=d_cstf[:])
        xha = [sb1.tile([128, XSPLIT], f16, tag=f"xh{h}a", name=f"xh{h}a")
               for h in range(2)]
        xb = sb1.tile([128, 2, NPX - XSPLIT], f16, tag="xb")
        for h in range(2):
            nc.scalar.dma_start(out=xha[h],
                                in_=d_x[:, h * NPX:h * NPX + XSPLIT])
        # both "b" chunks in one 2-range DMA (fewer DMAs -> fewer semaphore
        # pool collisions with the transpose stream)
        d_x_ap = d_x[:]
        nc.scalar.dma_start(
            out=xb, in_=bass.AP(tensor=d_x_ap.tensor,
                                offset=d_x_ap.offset + XSPLIT,
                                ap=[d_x_ap.ap[0], [NPX, 2],
                                    [1, NPX - XSPLIT]]))
        nc.scalar.dma_start(out=sidx, in_=d_idx[:])
        xt = sb1.tile([128, NJB, 2, 128], f16, tag="xt")
        nc.sync.dma_start(
            out=xt, in_=d_xt[:].rearrange("q (j c m) -> q j c m", j=NJB, c=2))

        bc = cstf[0:MID, 0:1]
        be = cstf[0:ENC, 1:2]
        wc0 = cst[:, C_WC0:C_WC0 + 64]
        wc1 = cst[:, C_WC1:C_WC1 + 64]
        ones = cst[0:ENC, C_ONES:C_ONES + 4]

        # warm the Exp activation table during load shadow
        scratch = sb1.tile([ENC, 1], f32, tag="scratch")
        nc.scalar.activation(out=scratch[:], in_=be,
                             func=mybir.ActivationFunctionType.Exp,
                             bias=be, scale=1.0)

        featd = sb1.tile([128, FROW * FPW], f16, tag="featd")
        nc.vector.memset(featd, 0.0)
        wkn = sb1.tile([112, 2048], f16, tag="wkn")
        nc.vector.memset(wkn[96:112, :], 0.0)
        wkTs = [None] * 4   # per conv tile [128, 4, 112] raw transposed exp
        wkT_x = [None] * 4  # transpose instr per conv tile
        exp_is = [None] * 4
        wnbs = [None] * NBLK   # per block normalized scatter feed [128, 100]

        def xsrc(h, n0, n):
            """x slice [n0, n0+n) of half h from the right chunk tile."""
            if n0 + n <= XSPLIT:
                return xha[h][:, n0:n0 + n]
            return xb[:, h, n0 - XSPLIT:n0 - XSPLIT + n]

        # ---- conv1 tile: 1x1 conv (256->64) + relu -> featd (both halves)
        def conv1_tile(nt):
            n0 = W + nt * 512
            n = min(512, 2240 - n0)
            pf = ps.tile([MID, 512], f32, tag="pf", bufs=2, name="pf")
            nc.tensor.matmul(pf[:, :n], wc0, xsrc(0, n0, n),
                             start=True, stop=False)
            nc.tensor.matmul(pf[:, :n], wc1, xsrc(1, n0, n),
                             start=False, stop=True)
            fp0 = n0 // W - 1
            nr = n // W
            src = pf[:, :n].rearrange("m (r w) -> m r w", w=W)
            fd1h = featd[0:64]
            dst1 = bass.AP(
                tensor=fd1h.tensor, offset=fd1h.offset + fp0 * FPW + 1,
                ap=[fd1h.ap[0], [FPW, nr], [1, W]],
            )
            nc.scalar.activation(out=dst1, in_=src,
                                 func=mybir.ActivationFunctionType.Relu,
                                 bias=bc, scale=1.0)
            fd2 = featd[64:128]
            if fp0 == 0:
                src2 = bass.AP(tensor=pf.tensor, offset=pf.offset + W,
                               ap=[pf.ap[0], [W, nr - 1], [1, W]])
                dst2 = bass.AP(tensor=fd2.tensor, offset=fd2.offset + 1,
                               ap=[fd2.ap[0], [FPW, nr - 1], [1, W]])
            else:
                src2 = bass.AP(tensor=pf.tensor, offset=pf.offset,
                               ap=[pf.ap[0], [W, nr], [1, W]])
                dst2 = bass.AP(tensor=fd2.tensor,
                               offset=fd2.offset + (fp0 - 1) * FPW + 1,
                               ap=[fd2.ap[0], [FPW, nr], [1, W]])
            nc.scalar.activation(out=dst2, in_=src2,
                                 func=mybir.ActivationFunctionType.Relu,
                                 bias=bc, scale=1.0)

        # ---- conv2: 6 matmuls per 8-row tile, + bias + exp -> wkn
        def conv2_mm(nt):
            h0 = nt * 8
            pw = ps.tile([ENC, 512], f32, tag="pw", bufs=1, name="pw")
            for j in range(3):
                rhs = bass.AP(
                    tensor=featd.tensor, offset=featd.offset + h0 * FPW + j,
                    ap=[featd.ap[0], [FPW, 8], [1, W]],
                )
                nc.tensor.matmul(pw[:], cst[:, C_WEP + j * ENC:C_WEP + (j + 1) * ENC],
                                 rhs, start=(j == 0), stop=False)
            fd1 = featd[0:64]
            for j in range(3):
                rhs = bass.AP(
                    tensor=fd1.tensor,
                    offset=fd1.offset + (h0 + 2) * FPW + j,
                    ap=[fd1.ap[0], [FPW, 8], [1, W]],
                )
                nc.tensor.matmul(pw[:], cst[0:64, C_WES + j * ENC:C_WES + (j + 1) * ENC],
                                 rhs, start=False, stop=(j == 2))
            return pw

        def conv2_exp(nt, pw):
            sl = slice(nt * 512, (nt + 1) * 512)
            e = nc.scalar.activation(out=wkn[0:ENC, sl], in_=pw[:],
                                     func=mybir.ActivationFunctionType.Exp,
                                     bias=be, scale=1.0)
            exp_is[nt] = e

        def wkt_x(nt):
            """XBAR transpose raw exp (wkn) -> wkT for conv tile nt."""
            sl = slice(nt * 512, (nt + 1) * 512)
            wkT = sbw.tile([128, 4, 112], f16, tag="wkT", bufs=4, name="wkT")
            wkTs[nt] = wkT
            wx = nc.sync.dma_start_transpose(out=wkT[:], in_=wkn[:, sl])
            # Tile does not dep-track XBAR transposes: manual edge
            add_dep_helper(wx.ins, exp_is[nt].ins, reason="xbar wkT reads wkn")
            wkT_x[nt] = wx
            return wx

        def tile_norm(nt, wx):
            """Post-transpose softmax normalization for conv tile nt: one
            [128,16] sums group + one reciprocal + 16 tensor_scalar_mul."""
            pa = ps.tile([128, 16], f32, tag="pa", bufs=1, name="pa")
            for b in range(4):
                t = nt * 4 + b
                nc.tensor.matmul(pa[:, b * 4:(b + 1) * 4],
                                 wkn[0:ENC, t * 128:(t + 1) * 128], ones,
                                 start=True, stop=True)
            rT = sbw.tile([128, 16], f32, tag="rT", bufs=2, name="rT")
            nc.vector.reciprocal(rT[:], pa[:])
            for b in range(4):
                t = nt * 4 + b
                wnb = sbw.tile([128, 100], f16, tag="wnb", bufs=12, name="wnb")
                wnbs[t] = wnb
                for p in range(4):
                    ts = nc.vector.tensor_scalar_mul(
                        wnb[:, p * 25:(p + 1) * 25],
                        wkTs[nt][:, b, p * 25:p * 25 + 25],
                        rT[:, b * 4 + p:b * 4 + p + 1])
                    if b == 0 and p == 0:
                        add_dep_helper(ts.ins, wx.ins,
                                       reason="norm reads xbar wkT")

        # ---- conv phase emission
        conv1_tile(0)
        conv1_tile(1)
        pw0 = conv2_mm(0)
        conv2_exp(0, pw0)
        conv1_tile(2)
        w0 = wkt_x(0)
        tile_norm(0, w0)
        pw1 = conv2_mm(1)
        conv2_exp(1, pw1)
        conv1_tile(3)
        w1 = wkt_x(1)
        tile_norm(1, w1)
        pw2 = conv2_mm(2)
        conv2_exp(2, pw2)
        conv1_tile(4)
        w2 = wkt_x(2)
        tile_norm(2, w2)
        pw3 = conv2_mm(3)
        conv2_exp(3, pw3)
        w3 = wkt_x(3)
        tile_norm(3, w3)

        # ---- reassembly: per block scatter; per pair one XBAR transpose,
        # 12 matmuls, contiguous fp16 evictions, grouped stores.
        sdst_reader = [None] * 6
        t2list = [None] * 8
        osegs = [None] * 4

        def scatter(t):
            g = t // 2
            if t % 2 == 0:
                sd = sbw.tile([128, 2, 1536], f16, tag="sdst", bufs=6, name="sd")
                t2list[g] = sd
            sd = t2list[g]
            sc = nc.gpsimd.local_scatter(
                out_ap=sd[:, t % 2, :], data_ap=wnbs[t][:],
                idxs_ap=sidx[:],
                channels=128, num_elems=1536, num_idxs=100,
            )
            if sdst_reader[g % 6] is not None:
                add_dep_helper(sc.ins, sdst_reader[g % 6].ins,
                               reason="WAR: scatter overwrites xbar-read sdst")
            return sc

        def transpose_pair(g, sc0, sc1):
            T2 = sbw.tile([128, 24, 128], f16, tag="T2", bufs=6, name="T2")
            tx = nc.sync.dma_start_transpose(out=T2[:], in_=t2list[g][:])
            add_dep_helper(tx.ins, sc0.ins, reason="xbar reads scatter0")
            add_dep_helper(tx.ins, sc1.ins, reason="xbar reads scatter1")
            sdst_reader[g % 4] = tx
            return T2, tx

        def reassemble(t, T2, tx):
            for ch in range(2):
                po = ps.tile([128, 512], f32, tag="po", bufs=4, name="po")
                for dj in range(3):
                    pan = (t % 2) * 12 + dj * 4
                    mm_i = nc.tensor.matmul(
                        po[:], xt[:, t + dj, ch, :], T2[:, pan:pan + 4, :],
                        start=(dj == 0), stop=(dj == 2),
                    )
                    if ch == 0 and dj == 0:
                        add_dep_helper(mm_i.ins, tx.ins, reason="PE reads xbar T")
                if t % 8 == 0 and ch == 0:
                    osegs[(t // 8)] = sbw.tile([128, 16, 512], f16, tag="oseg",
                                               bufs=2, name="oseg")
                dst = osegs[t // 8][:, (t % 8) * 2 + ch, :]
                if ch == 0:
                    nc.scalar.activation(out=dst, in_=po[:],
                                         func=mybir.ActivationFunctionType.Copy,
                                         scale=1.0)
                else:
                    nc.vector.tensor_copy(dst, po[:])
            last = mm_i
            if t % 8 == 7:
                k = t // 8
                nc.scalar.dma_start(
                    out=d_out[:, k * 8192:(k + 1) * 8192],
                    in_=osegs[k][:],
                )
            return last

        # software-pipelined emission: pair g+1's scatters+transpose are
        # emitted before pair g's matmuls. wkT2/wkT3 are sandwiched into the
        # T2 stream with order pins so the scheduler cannot misplace them.
        pend = {}
        last_pe = [None] * 6   # T2 slot -> last PE reader
        sc0 = scatter(0)
        sc1 = scatter(1)
        pend[0] = transpose_pair(0, sc0, sc1)
        # early wkT transposes must precede the T2 stream on the sync ring
        add_dep_helper(pend[0][1].ins, w0.ins, reason="order T2x0 after wkT0")
        add_dep_helper(pend[0][1].ins, w1.ins, reason="order T2x0 after wkT1")
        # sandwich wkT2 between T2x0 and T2x1, wkT3 between T2x2 and T2x3
        add_dep_helper(w2.ins, pend[0][1].ins, reason="order wkT2 after T2x0")
        for g in range(8):
            if g + 1 < 8:
                sc0 = scatter(2 * g + 2)
                sc1 = scatter(2 * g + 3)
                pend[g + 1] = transpose_pair(g + 1, sc0, sc1)
                if g + 1 == 1:
                    add_dep_helper(pend[1][1].ins, w2.ins,
                                   reason="order T2x1 after wkT2")
                if g + 1 == 2:
                    add_dep_helper(w3.ins, pend[2][1].ins,
                                   reason="order wkT3 after T2x2")
                if g + 1 == 3:
                    add_dep_helper(pend[3][1].ins, w3.ins,
                                   reason="order T2x3 after wkT3")
                if last_pe[(g + 1) % 6] is not None:
                    add_dep_helper(pend[g + 1][1].ins, last_pe[(g + 1) % 6].ins,
                                   reason="WAR: xbar overwrites PE-read T2")
            T2, tx = pend[g]
            reassemble(2 * g, T2, tx)
            last_pe[g % 6] = reassemble(2 * g + 1, T2, tx)

    nc.compile()
    return nc


def _host_prep(x, W_comp, b_comp, W_enc, b_enc):
    """Build per-core input maps (layout/dtype prep only)."""
    idxs = _build_idxs()
    cst = np.zeros((128, C_TOT), np.float16)
    cst[0:128, C_WC0:C_WC0 + 64] = W_comp.T[0:128]
    cst[0:128, C_WC1:C_WC1 + 64] = W_comp.T[128:256]
    for j in range(3):
        cst[0:64, C_WEP + j * ENC:C_WEP + (j + 1) * ENC] = W_enc[:, :, 0, j].T
        cst[64:128, C_WEP + j * ENC:C_WEP + (j + 1) * ENC] = W_enc[:, :, 1, j].T
        cst[0:64, C_WES + j * ENC:C_WES + (j + 1) * ENC] = W_enc[:, :, 2, j].T
    for p in range(4):
        cst[C_ONES * 0 + p * 25:(p + 1) * 25, C_ONES + p] = 1.0
        cst[p, C_E4 + p * 25:C_E4 + (p + 1) * 25] = 1.0
    cstf = np.zeros((128, 2), np.float32)
    cstf[0:MID, 0] = b_comp
    cstf[0:ENC, 1] = b_enc

    xp = np.pad(x, ((0, 0), (0, 0), (2, 2), (0, 0)))   # (B, C, 68, 64)
    in_maps = []
    for core in range(8):
        b, half = core // 2, core % 2
        r0 = 32 * half
        xs = np.ascontiguousarray(
            xp[b, :, r0:r0 + NROW, :].reshape(C, NPX)).astype(np.float16)
        xflat = np.ascontiguousarray(xs.reshape(2, 128, NPX).transpose(1, 0, 2)
                                     ).reshape(128, 2 * NPX)
        xtc = np.ascontiguousarray(
            xs.reshape(2, 128, NJB, 128).transpose(3, 2, 0, 1)
        ).reshape(128, NJB * 2 * 128)
        in_maps.append(dict(x=xflat, xt=xtc, cst=cst, cstf=cstf, idx=idxs))
    return in_maps


def _gather(res):
    """Assemble full (B, C, 128, 128) fp32 output from per-core raw stores.

    Per core: out [128, 32, 512] f16 where free = (t, ch2, p, rt, w) with
    po free = (p, rt*64+w). Output pixel-shuffle: input row i = 2t+rt of the
    half; out rows 2i + (4w+p)//128, col (4w+p) % 128.
    """
    out = np.empty((B, C, 128, 128), np.float32)
    for core in range(8):
        b, half = core // 2, core % 2
        a = res.results[core]["out"].astype(np.float32)
        a = a.reshape(128, 16, 2, 4, 2, 64)      # (cp, t, c2, p, rt, w)
        a = a.transpose(1, 4, 2, 0, 5, 3)        # (t, rt, c2, cp, w, p)
        a = a.reshape(16, 2, 256, 2, 128)        # (t, rt, ch, rowbit, col)
        a = a.transpose(2, 0, 1, 3, 4).reshape(256, 64, 128)
        out[b, :, 64 * half:64 * (half + 1), :] = a
    return out


def kernel(x, W_comp, b_comp, W_enc, b_enc):
    x = np.asarray(x, np.float32)
    W_comp = np.asarray(W_comp, np.float32)
    b_comp = np.asarray(b_comp, np.float32)
    W_enc = np.asarray(W_enc, np.float32)
    b_enc = np.asarray(b_enc, np.float32)

    if "nc" not in _CACHE:
        _CACHE["nc"] = _build_nc()
    nc = _CACHE["nc"]

    in_maps = _host_prep(x, W_comp, b_comp, W_enc, b_enc)
    res = run_bass_kernel_spmd(nc, in_maps, core_ids=list(range(8)))
    return _gather(res)


if __name__ == "__main__":
    d = np.load("/tmp/carafe_ref.npz")
    expected = d["expected"]
    out = kernel(**{k: d[k] for k in ["x", "W_comp", "b_comp", "W_enc", "b_enc"]})
    err = np.abs(out - expected)
    scale = np.abs(expected).max()
    print(f"absmax err: {err.max():.4e}  rel: {err.max()/scale:.4e}")
